# revision 15
# baseline (speedup 1.0000x reference)
"""Trainium2 Bass kernel for modulated multi-head attention (q=k=v variant).

v2 — restructured from the v1 baseline using HW calibration:
  * All weight modulation/demodulation (style matvec, w*style, rsqrt demod,
    for both k- and o-projections) is precomputed on HOST in fp32 and shipped
    as per-batch effective weight matrices (bf16). The device only runs:
      kqvT = wk_eff @ x^T              [F-part, N]   (q/k source, demodulated)
      kqv_v = x @ wk_eff^T             [N-part, F]   (v source, demodulated)
      per head h: S_h = q_h q_h^T/8 ; E=exp(S) with fused rowsum (accum_out)
      outT_h = v_h^T E_h               (attnv, PSUM-accumulated over m-blocks)
      aT = outT * (1/rowsum) broadcast (DRAM-bounce broadcast as in v1)
      y = aT^T @ wo_eff                (pair-merged K=128 projection)
  * HW calibration findings applied:
      - consecutive matmuls must not alternate PE tile configs
        (tile_position / stationary size); poison costs ~4us per switch.
        All matmul streams are batched per config (scores h0-batch, h1-batch,
        attnv h0/h1 batches per half-pair).
      - accumulating matmul groups must alternate PSUM banks between
        consecutive instructions (same-bank back-to-back is ~5x slow);
        kqvT / kqv_v / yproj groups are emitted pairwise bank-interleaved.
      - exp (FD=1024, accum_out) sustains ~1.05us when paced cross-engine
        with rotating PSUM sources; scores tiles rotate 3 slots.
  * exp order per pair: all 8 h0-exps (first half), then 8 h1-exps — this
    makes every PE stream a clean per-config batch.
  * PSUM budget (8 banks): "sc" [128,1024]x3 slots (6 banks; scores tiles and
    scratch for kqvT/kqv_v/yproj groups) + accA/accB [128,512] (2 banks,
    attnv accumulators: h0 rows 0:64 cfg (0,0), h1 rows 64:128 cfg (0,64)).

Sharding: data-parallel over batch B=8, one batch element per NeuronCore.
"""

import sys

if "/opt/trn_rl_repo" not in sys.path:
    sys.path.insert(0, "/opt/trn_rl_repo")

from contextlib import ExitStack

import numpy as np

import concourse.bass as bass
import concourse.bacc as bacc
import concourse.mybir as mybir
import concourse.tile as tile
from concourse.bass_utils import run_bass_kernel_spmd

P = 128          # partitions
F = 512          # hidden dim
C4 = F // P      # 4 feature chunks of 128
N = 1024         # tokens
NB = N // P      # 8 token blocks
H = 8            # heads
D = 64           # head dim
B = 8            # batch (one per core)
SCALE = 1.0 / 8.0   # 1/sqrt(D)
EPS = 1e-8

F32 = mybir.dt.float32
BF16 = mybir.dt.bfloat16


def _bcast(ap_1d, parts):
    """Partition-broadcast read AP for a 1-D DRAM AP."""
    return bass.AP(
        tensor=ap_1d.tensor,
        offset=ap_1d.offset,
        ap=[[0, parts]] + [list(d) for d in ap_1d.ap],
    )


def _emit(nc, loop_reps=0, lvl=4):
    xT = nc.dram_tensor("xT", [F, N], BF16, kind="ExternalInput")
    wkT = nc.dram_tensor("wkT", [F, F], BF16, kind="ExternalInput")
    woT = nc.dram_tensor("woT", [F, F], BF16, kind="ExternalInput")
    y = nc.dram_tensor("y", [N, F], F32, kind="ExternalOutput")

    with tile.TileContext(nc) as tc:
        if loop_reps:
            with tc.For_i(0, loop_reps, 1):
                _emit_body(nc, tc, xT, wkT, woT, y, lvl=lvl)
        else:
            _emit_body(nc, tc, xT, wkT, woT, y, lvl=lvl)


def _emit_body(nc, tc, xT, wkT, woT, y, lvl=4):
    f32 = F32
    Exp = mybir.ActivationFunctionType.Exp
    MULT = mybir.AluOpType.mult

    with ExitStack() as ctx:
        persist = ctx.enter_context(tc.tile_pool(name="persist", bufs=1))
        dram = ctx.enter_context(tc.tile_pool(name="dram", bufs=2, space="DRAM"))
        psum = ctx.enter_context(tc.tile_pool(name="psum", bufs=1, space="PSUM"))
        att = ctx.enter_context(tc.tile_pool(name="att", bufs=1))
        attrs = ctx.enter_context(tc.tile_pool(name="attrs", bufs=2))

        # ---- persistent SBUF tiles ----
        xT_sb = persist.tile([P, C4, N], BF16)
        wk_sb = persist.tile([P, C4, F], BF16)
        wo_sb = persist.tile([P, C4, F], BF16)
        kqvT = persist.tile([P, C4, N], BF16)
        kqv_v = persist.tile([P, NB, F], BF16)
        aT = persist.tile([P, C4, N], BF16)
        y_acc = persist.tile([P, NB, F], f32)

        # exp-table prewarm while input DMAs stream
        warm = persist.tile([1, 1], f32)
        nc.vector.memset(warm, 1.0)
        nc.scalar.activation(out=warm, in_=warm, func=Exp, scale=1.0)

        # ---- input DMAs ----
        xT_r = xT.rearrange("(c p) n -> p c n", p=P)
        for nh in range(2):
            nc.sync.dma_start(out=xT_sb[:, :, nh * F : (nh + 1) * F],
                              in_=xT_r[:, :, nh * F : (nh + 1) * F])
        nc.gpsimd.dma_start(out=wk_sb, in_=wkT.rearrange("(c p) o -> p c o", p=P))

        def sc_tile():
            return psum.tile([P, N], f32, tag="sc", bufs=2, name="sc")

        # acc tiles are long-lived across the pair pipeline: allocate the four
        # tags once per pair via pair_state; between pairs the same banks
        # rotate through kqvT/kqv_v scratch duty (pair 0) and attnv duty.
        pair_state = {}

        # ---- kqvT chunk ob on two scratch psum banks (2-MM steps) ----
        def kqvT_steps(ob, pa, pb):
            for c in range(C4):
                def step(c=c):
                    nc.tensor.matmul(
                        pa, wk_sb[:, c, ob * P : (ob + 1) * P],
                        xT_sb[:, c, 0:F],
                        start=(c == 0), stop=(c == C4 - 1))
                    nc.tensor.matmul(
                        pb, wk_sb[:, c, ob * P : (ob + 1) * P],
                        xT_sb[:, c, F:N],
                        start=(c == 0), stop=(c == C4 - 1))
                yield step
            def evac():
                nc.vector.tensor_copy(out=kqvT[:, ob, 0:F], in_=pa)
                nc.vector.tensor_copy(out=kqvT[:, ob, F:N], in_=pb)
            yield evac

        # ---- kqv_v blocks nb0, nb0+1 on two scratch banks ----
        def kqv_v_steps(nb0, pa, pb):
            for c in range(C4):
                def step(c=c):
                    nc.tensor.matmul(
                        pa, xT_sb[:, c, nb0 * P : (nb0 + 1) * P],
                        wk_sb[:, c, :],
                        start=(c == 0), stop=(c == C4 - 1))
                    nc.tensor.matmul(
                        pb, xT_sb[:, c, (nb0 + 1) * P : (nb0 + 2) * P],
                        wk_sb[:, c, :],
                        start=(c == 0), stop=(c == C4 - 1))
                yield step
            def evac():
                nc.vector.tensor_copy(out=kqv_v[:, nb0, :], in_=pa)
                nc.vector.tensor_copy(out=kqv_v[:, nb0 + 1, :], in_=pb)
            yield evac

        # ---- attnv 2-MM step for (pair, head-half, m-block) ----
        def attnv_step(pc, hh, mb):
            h = 2 * pc + hh
            st = pair_state[pc]
            E = st["E0" if hh == 0 else "E1"]
            lo, hi = (0, D) if hh == 0 else (D, P)
            kw = {} if hh == 0 else {"tile_position": (0, 64)}
            keys = ("accA", "accB") if hh == 0 else ("accC", "accD")
            def step():
                for nh, key in ((0, keys[0]), (1, keys[1])):
                    nc.tensor.matmul(
                        st[key][lo:hi, :],
                        kqv_v[:, mb, h * D : (h + 1) * D],
                        E[:, mb, nh * F : (nh + 1) * F],
                        start=(mb == 0), stop=(mb == NB - 1), **kw)
            return step

        # ---- y projection partial: 2 blocks per sc slot, merged add ----
        def ypartial_steps(pc, with_dma):
            for nb0 in range(0, NB, 2):
                def step(nb0=nb0):
                    pt = sc_tile()
                    for k in range(2):
                        nc.tensor.matmul(
                            pt[:, k * F : (k + 1) * F],
                            aT[:, pc, (nb0 + k) * P : (nb0 + k + 1) * P],
                            wo_sb[:, pc, :],
                            start=True, stop=True)
                    if pc == 0:
                        nc.vector.tensor_copy(out=y_acc[:, nb0 : nb0 + 2, :],
                                              in_=pt)
                    else:
                        nc.vector.tensor_add(out=y_acc[:, nb0 : nb0 + 2, :],
                                             in0=y_acc[:, nb0 : nb0 + 2, :],
                                             in1=pt)
                    if with_dma:
                        eng = nc.sync if nb0 % 4 == 0 else nc.gpsimd
                        eng.dma_start(
                            out=y.rearrange("(b p) f -> p b f", p=P)[:, nb0 : nb0 + 2, :],
                            in_=y_acc[:, nb0 : nb0 + 2, :])
                yield step

        def emit_pair_finish(pc):
            """rowsum reciprocal -> DRAM-bounce broadcast (launched early so
            the broadcast lands well before the evac TTs need it)."""
            st = pair_state[pc]
            rows0, rows1 = st["rows0"], st["rows1"]
            nc.vector.reciprocal(out=rows0, in_=rows0)
            nc.vector.reciprocal(out=rows1, in_=rows1)
            d_r0 = dram.tile([N], f32, tag="d_r0")
            d_r1 = dram.tile([N], f32, tag="d_r1")
            nc.sync.dma_start(out=d_r0.rearrange("(c p) -> p c", p=P), in_=rows0)
            nc.gpsimd.dma_start(out=d_r1.rearrange("(c p) -> p c", p=P), in_=rows1)
            rs_b = attrs.tile([P, N], f32, tag="rs_b")
            nc.sync.dma_start(out=rs_b[0:D, :], in_=_bcast(d_r0, D))
            nc.gpsimd.dma_start(out=rs_b[D:P, :], in_=_bcast(d_r1, D))
            st["rs_b"] = rs_b

        def emit_evac(pc):
            st = pair_state[pc]
            rs_b = st["rs_b"]
            for nh in range(2):
                sl = slice(nh * F, (nh + 1) * F)
                accLo = st["accA" if nh == 0 else "accB"]
                accHi = st["accC" if nh == 0 else "accD"]
                nc.vector.tensor_tensor(aT[0:D, pc, sl], accLo[0:D, :],
                                        rs_b[0:D, sl], MULT)
                nc.vector.tensor_tensor(aT[D:P, pc, sl], accHi[D:P, :],
                                        rs_b[D:P, sl], MULT)

        def scores_slot(pc, hh, mb, extras):
            """One pipeline slot: 2 scores MMs (one config), up to a few
            extra 2-MM steps from other streams, then the exp."""
            st = pair_state[pc]
            E = st["E0" if hh == 0 else "E1"]
            rows = st["rows0" if hh == 0 else "rows1"]
            lo, hi = (0, D) if hh == 0 else (D, P)
            kw = {} if hh == 0 else {"tile_position": (64, 0)}
            s = sc_tile()
            for nh in range(2):
                nc.tensor.matmul(
                    s[:, nh * F : (nh + 1) * F],
                    kqvT[lo:hi, pc, mb * P : (mb + 1) * P],
                    kqvT[lo:hi, pc, nh * F : (nh + 1) * F],
                    start=True, stop=True, **kw)
            for fn in extras:
                fn()
            nc.scalar.activation(out=E[:, mb, :], in_=s, func=Exp,
                                 scale=SCALE, accum_out=rows[:, mb : mb + 1])

        def take(it, n):
            out = []
            for _ in range(n):
                nxt = next(it, None)
                if nxt is None:
                    break
                out.append(nxt)
            return out

        def chain(*gens):
            for g in gens:
                yield from g

        # =================== prologue ===================
        # scratch psum for kqvT/kqv_v during pair 0: the (not yet used)
        # attnv accumulator banks.
        scrA = psum.tile([P, F], f32, tag="accA", name="scrA")
        scrB = psum.tile([P, F], f32, tag="accB", name="scrB")
        scrC = psum.tile([P, F], f32, tag="accC", name="scrC")
        scrD = psum.tile([P, F], f32, tag="accD", name="scrD")
        for step in kqvT_steps(0, scrA, scrB):
            step()

        def alloc_accs(pc):
            st = pair_state[pc]
            st["accA"] = psum.tile([P, F], f32, tag="accA", name="accA")
            st["accB"] = psum.tile([P, F], f32, tag="accB", name="accB")
            st["accC"] = psum.tile([P, F], f32, tag="accC", name="accC")
            st["accD"] = psum.tile([P, F], f32, tag="accD", name="accD")

        # =================== pair loop ===================
        for pc in range(H // 2):
            E0 = att.tile([P, NB, N], BF16, tag="E0", bufs=2)
            E1 = att.tile([P, NB, N], BF16, tag="E1", bufs=2)
            rows0 = attrs.tile([P, NB], f32, tag="rows0")
            rows1 = attrs.tile([P, NB], f32, tag="rows1")
            pair_state[pc] = dict(E0=E0, E1=E1, rows0=rows0, rows1=rows1)

            if pc == 0:
                # extras for pair 0: kqvT chunks 1-3 + kqv_v on scratch banks.
                ex_h0 = chain(kqvT_steps(1, scrC, scrD),
                              kqvT_steps(2, scrA, scrB),
                              kqvT_steps(3, scrC, scrD))
                ex_h1 = chain(kqv_v_steps(0, scrA, scrB),
                              kqv_v_steps(2, scrC, scrD),
                              kqv_v_steps(4, scrA, scrB),
                              kqv_v_steps(6, scrC, scrD))
                n0, n1 = 2, 3
            elif pc == 1:
                # both halves of pair 0's attnv run here (kqv_v only became
                # fully available at the end of pair 0)
                alloc_accs(0)
                ex_h0 = chain(iter([attnv_step(0, 0, mb) for mb in range(NB)]),
                              iter([attnv_step(0, 1, mb) for mb in range(NB)]))
                ex_h1 = iter([attnv_step(1, 0, mb) for mb in range(NB)])
                n0, n1 = 2, 2
            else:
                ex_h0 = iter([attnv_step(pc - 1, 1, mb) for mb in range(NB)])
                ex_h1 = chain(
                    iter([attnv_step(pc, 0, mb) for mb in range(NB)]),
                    ypartial_steps(pc - 2, False))
                n0, n1 = 2, 2

            for mb in range(NB):
                scores_slot(pc, 0, mb, take(ex_h0, n0))
                if pc >= 1 and mb == 0:
                    emit_pair_finish(pc - 1)
            for fn in take(ex_h0, 99):
                fn()
            if pc >= 1:
                emit_evac(pc - 1)
                alloc_accs(pc)

            for mb in range(NB):
                scores_slot(pc, 1, mb, take(ex_h1, n1))
                if pc == 0 and mb == 3:
                    nc.gpsimd.dma_start(
                        out=wo_sb, in_=woT.rearrange("(c p) o -> p c o", p=P))
            for fn in take(ex_h1, 99):
                fn()

        # =================== tail ===================
        pc = H // 2 - 1
        for mb in range(NB):
            attnv_step(pc, 1, mb)()
        emit_pair_finish(pc)
        for step in ypartial_steps(2, False):
            step()
        emit_evac(pc)
        for step in ypartial_steps(pc, True):
            step()


_NC_CACHE = None


def build_nc():
    global _NC_CACHE
    if _NC_CACHE is None:
        nc = bacc.Bacc(trn_type="TRN2")
        _emit(nc)
        nc.finalize()
        _NC_CACHE = nc
    return _NC_CACHE


def _eff_weightT(weight, style):
    """Host: modulated+demodulated weight, transposed, bf16.
    weight [O, I] fp32, style [I] fp32 -> [I, O] bf16."""
    import ml_dtypes
    w = weight * style[None, :]
    w = w * (1.0 / np.sqrt((w * w).sum(axis=1) + EPS))[:, None]
    return np.ascontiguousarray(w.T.astype(ml_dtypes.bfloat16))


def make_in_maps(x, s, k_weight, k_aff_w, k_aff_b, o_weight, o_aff_w, o_aff_b):
    import ml_dtypes
    f = np.float32
    bf = ml_dtypes.bfloat16
    x = np.asarray(x, f)
    s = np.asarray(s, f)
    k_weight = np.asarray(k_weight, f)
    k_aff_w = np.asarray(k_aff_w, f)
    k_aff_b = np.asarray(k_aff_b, f)
    o_weight = np.asarray(o_weight, f)
    o_aff_w = np.asarray(o_aff_w, f)
    o_aff_b = np.asarray(o_aff_b, f)
    in_maps = []
    for b in range(B):
        style_k = s[b] @ k_aff_w.T + k_aff_b
        style_o = s[b] @ o_aff_w.T + o_aff_b
        in_maps.append({
            "xT": np.ascontiguousarray(x[b].T.astype(bf)),
            "wkT": _eff_weightT(k_weight, style_k),
            "woT": _eff_weightT(o_weight, style_o),
        })
    return in_maps


def kernel(x, s, k_weight, k_aff_w, k_aff_b, o_weight, o_aff_w, o_aff_b):
    assert x.shape == (B, N, F), x.shape
    nc = build_nc()
    in_maps = make_in_maps(x, s, k_weight, k_aff_w, k_aff_b,
                           o_weight, o_aff_w, o_aff_b)
    res = run_bass_kernel_spmd(nc, in_maps, list(range(B)))
    return np.stack([res.results[b]["y"] for b in range(B)], axis=0)


# revision 16
# speedup vs baseline: 1.1670x; 1.1670x over previous
"""Trainium2 Bass kernel for modulated multi-head attention (q=k=v variant).

v2 — restructured from the v1 baseline using HW calibration:
  * All weight modulation/demodulation (style matvec, w*style, rsqrt demod,
    for both k- and o-projections) is precomputed on HOST in fp32 and shipped
    as per-batch effective weight matrices (bf16). The device only runs:
      kqvT = wk_eff @ x^T              [F-part, N]   (q/k source, demodulated)
      kqv_v = x @ wk_eff^T             [N-part, F]   (v source, demodulated)
      per head h: S_h = q_h q_h^T/8 ; E=exp(S) with fused rowsum (accum_out)
      outT_h = v_h^T E_h               (attnv, PSUM-accumulated over m-blocks)
      aT = outT * (1/rowsum) broadcast (DRAM-bounce broadcast as in v1)
      y = aT^T @ wo_eff                (pair-merged K=128 projection)
  * HW calibration findings applied:
      - consecutive matmuls must not alternate PE tile configs
        (tile_position / stationary size); poison costs ~4us per switch.
        All matmul streams are batched per config (scores h0-batch, h1-batch,
        attnv h0/h1 batches per half-pair).
      - accumulating matmul groups must alternate PSUM banks between
        consecutive instructions (same-bank back-to-back is ~5x slow);
        kqvT / kqv_v / yproj groups are emitted pairwise bank-interleaved.
      - exp (FD=1024, accum_out) sustains ~1.05us when paced cross-engine
        with rotating PSUM sources; scores tiles rotate 3 slots.
  * exp order per pair: all 8 h0-exps (first half), then 8 h1-exps — this
    makes every PE stream a clean per-config batch.
  * PSUM budget (8 banks): "sc" [128,1024]x3 slots (6 banks; scores tiles and
    scratch for kqvT/kqv_v/yproj groups) + accA/accB [128,512] (2 banks,
    attnv accumulators: h0 rows 0:64 cfg (0,0), h1 rows 64:128 cfg (0,64)).

Sharding: data-parallel over batch B=8, one batch element per NeuronCore.
"""

import sys

if "/opt/trn_rl_repo" not in sys.path:
    sys.path.insert(0, "/opt/trn_rl_repo")

from contextlib import ExitStack

import numpy as np

import concourse.bass as bass
import concourse.bacc as bacc
import concourse.mybir as mybir
import concourse.tile as tile
from concourse.bass_utils import run_bass_kernel_spmd

P = 128          # partitions
F = 512          # hidden dim
C4 = F // P      # 4 feature chunks of 128
N = 1024         # tokens
NB = N // P      # 8 token blocks
H = 8            # heads
D = 64           # head dim
B = 8            # batch (one per core)
SCALE = 1.0 / 8.0   # 1/sqrt(D)
EPS = 1e-8

F32 = mybir.dt.float32
BF16 = mybir.dt.bfloat16


def _bcast(ap_1d, parts):
    """Partition-broadcast read AP for a 1-D DRAM AP."""
    return bass.AP(
        tensor=ap_1d.tensor,
        offset=ap_1d.offset,
        ap=[[0, parts]] + [list(d) for d in ap_1d.ap],
    )


def _emit(nc, loop_reps=0, lvl=4):
    xT = nc.dram_tensor("xT", [F, N], BF16, kind="ExternalInput")
    wkT = nc.dram_tensor("wkT", [F, F], BF16, kind="ExternalInput")
    woT = nc.dram_tensor("woT", [F, F], BF16, kind="ExternalInput")
    y = nc.dram_tensor("y", [N, F], F32, kind="ExternalOutput")

    with tile.TileContext(nc) as tc:
        if loop_reps:
            with tc.For_i(0, loop_reps, 1):
                _emit_body(nc, tc, xT, wkT, woT, y, lvl=lvl)
        else:
            _emit_body(nc, tc, xT, wkT, woT, y, lvl=lvl)


def _emit_body(nc, tc, xT, wkT, woT, y, lvl=4):
    f32 = F32
    Exp = mybir.ActivationFunctionType.Exp
    MULT = mybir.AluOpType.mult

    with ExitStack() as ctx:
        persist = ctx.enter_context(tc.tile_pool(name="persist", bufs=1))
        dram = ctx.enter_context(tc.tile_pool(name="dram", bufs=2, space="DRAM"))
        psum = ctx.enter_context(tc.tile_pool(name="psum", bufs=1, space="PSUM"))
        att = ctx.enter_context(tc.tile_pool(name="att", bufs=1))
        attrs = ctx.enter_context(tc.tile_pool(name="attrs", bufs=2))

        # ---- persistent SBUF tiles ----
        xT_sb = persist.tile([P, C4, N], BF16)
        wk_sb = persist.tile([P, C4, F], BF16)
        wo_sb = persist.tile([P, C4, F], BF16)
        kqvT = persist.tile([P, C4, N], BF16)
        kqv_v = persist.tile([P, NB, F], BF16)
        aT = persist.tile([P, C4, N], BF16)
        y_acc = persist.tile([P, NB, F], f32)

        # exp-table prewarm while input DMAs stream
        warm = persist.tile([1, 1], f32)
        nc.vector.memset(warm, 1.0)
        nc.scalar.activation(out=warm, in_=warm, func=Exp, scale=1.0)

        # ---- input DMAs ----
        xT_r = xT.rearrange("(c p) n -> p c n", p=P)
        for nh in range(2):
            nc.sync.dma_start(out=xT_sb[:, :, nh * F : (nh + 1) * F],
                              in_=xT_r[:, :, nh * F : (nh + 1) * F])
        nc.gpsimd.dma_start(out=wk_sb, in_=wkT.rearrange("(c p) o -> p c o", p=P))

        def sc_tile():
            return psum.tile([P, N], f32, tag="sc", bufs=2, name="sc")

        # ---- kqvT chunk: features chunk ob (pair ob), both n-halves ----
        def emit_kqvT_chunk(ob):
            pt = sc_tile()
            for c in range(C4):
                for nh in range(2):
                    nc.tensor.matmul(
                        pt[:, nh * F : (nh + 1) * F],
                        wk_sb[:, c, ob * P : (ob + 1) * P],
                        xT_sb[:, c, nh * F : (nh + 1) * F],
                        start=(c == 0), stop=(c == C4 - 1))
            nc.vector.tensor_copy(out=kqvT[:, ob, :], in_=pt)

        # ---- kqv_v: two n-blocks at a time, banks interleaved ----
        def emit_kqv_v_pair(nb0):
            pt = sc_tile()
            for c in range(C4):
                for k in range(2):
                    nc.tensor.matmul(
                        pt[:, k * F : (k + 1) * F],
                        xT_sb[:, c, (nb0 + k) * P : (nb0 + k + 1) * P],
                        wk_sb[:, c, :],
                        start=(c == 0), stop=(c == C4 - 1))
            nc.vector.tensor_copy(out=kqv_v[:, nb0, :], in_=pt[:, 0:F])
            nc.vector.tensor_copy(out=kqv_v[:, nb0 + 1, :], in_=pt[:, F:N])

        # ---- y projection partial for pair pc (K=128, two blocks/slot) ----
        def emit_ypartial(pc, nbs, with_dma):
            for i in range(0, len(nbs), 2):
                pt = sc_tile()
                for k in range(2):
                    nb = nbs[i + k]
                    nc.tensor.matmul(
                        pt[:, k * F : (k + 1) * F],
                        aT[:, pc, nb * P : (nb + 1) * P],
                        wo_sb[:, pc, :],
                        start=True, stop=True)
                for k in range(2):
                    nb = nbs[i + k]
                    sl = pt[:, k * F : (k + 1) * F]
                    if pc == 0:
                        nc.vector.tensor_copy(out=y_acc[:, nb, :], in_=sl)
                    else:
                        nc.vector.tensor_add(out=y_acc[:, nb, :],
                                             in0=y_acc[:, nb, :], in1=sl)
                    if with_dma:
                        eng = nc.sync if nb % 2 == 0 else nc.gpsimd
                        eng.dma_start(out=y[nb * P : (nb + 1) * P, :],
                                      in_=y_acc[:, nb, :])

        emit_kqvT_chunk(0)

        # per-pair state carried across the pair loop
        pair_state = {}

        def emit_scores_batch(pc, hh, E, rows, mbs):
            """Batch of scores MMs + exps for head-half hh (0: rows 0:64,
            1: rows 64:128 with tile_position) of pair pc."""
            lo, hi = (0, D) if hh == 0 else (D, P)
            kw = {} if hh == 0 else {"tile_position": (64, 0)}
            for mb in mbs:
                s = sc_tile()
                for nh in range(2):
                    nc.tensor.matmul(
                        s[:, nh * F : (nh + 1) * F],
                        kqvT[lo:hi, pc, mb * P : (mb + 1) * P],
                        kqvT[lo:hi, pc, nh * F : (nh + 1) * F],
                        start=True, stop=True, **kw)
                nc.scalar.activation(out=E[:, mb, :], in_=s, func=Exp,
                                     scale=SCALE,
                                     accum_out=rows[:, mb : mb + 1])

        def emit_attnv_batch(pc, hh, mbs):
            """attnv for head 2*pc+hh: stationary v cols; h0 writes its own
            acc pair at partitions 0:64, h1 writes another acc pair at
            partitions 64:128 via col tile_position (batched config)."""
            h = 2 * pc + hh
            st = pair_state[pc]
            E = st["E0" if hh == 0 else "E1"]
            lo, hi = (0, D) if hh == 0 else (D, P)
            kw = {} if hh == 0 else {"tile_position": (0, 64)}
            accs = (st["accA"], st["accB"]) if hh == 0 else (st["accC"], st["accD"])
            for mb in mbs:
                first, last = mb == mbs[0], mb == mbs[-1]
                for nh, acc in ((0, accs[0]), (1, accs[1])):
                    nc.tensor.matmul(
                        acc[lo:hi, :],
                        kqv_v[:, mb, h * D : (h + 1) * D],
                        E[:, mb, nh * F : (nh + 1) * F],
                        start=first, stop=last, **kw)

        def emit_pair_finish(pc):
            """rowsum reciprocal -> DRAM-bounce broadcast -> normalize-evac
            of the attnv accumulators into aT[:, pc, :]."""
            st = pair_state[pc]
            rows0, rows1 = st["rows0"], st["rows1"]
            nc.vector.reciprocal(out=rows0, in_=rows0)
            nc.vector.reciprocal(out=rows1, in_=rows1)
            d_r0 = dram.tile([N], f32, tag="d_r0")
            d_r1 = dram.tile([N], f32, tag="d_r1")
            nc.sync.dma_start(out=d_r0.rearrange("(c p) -> p c", p=P), in_=rows0)
            nc.gpsimd.dma_start(out=d_r1.rearrange("(c p) -> p c", p=P), in_=rows1)
            rs_b = attrs.tile([P, N], f32, tag="rs_b")
            nc.sync.dma_start(out=rs_b[0:D, :], in_=_bcast(d_r0, D))
            nc.gpsimd.dma_start(out=rs_b[D:P, :], in_=_bcast(d_r1, D))
            st["rs_b"] = rs_b

        def emit_evac(pc, last):
            st = pair_state[pc]
            rs_b = st["rs_b"]
            for nh in range(2):
                sl = slice(nh * F, (nh + 1) * F)
                accLo = st["accA" if nh == 0 else "accB"]
                accHi = st["accC" if nh == 0 else "accD"]
                nc.vector.tensor_tensor(aT[0:D, pc, sl], accLo[0:D, :],
                                        rs_b[0:D, sl], MULT)
                nc.vector.tensor_tensor(aT[D:P, pc, sl], accHi[D:P, :],
                                        rs_b[D:P, sl], MULT)
                if last:
                    emit_ypartial(pc, [nh * 4 + j for j in range(4)],
                                  with_dma=True)

        # =================== pair loop ===================
        for pc in range(H // 2):
            E0 = att.tile([P, NB, N], BF16, tag="E0", bufs=2)
            E1 = att.tile([P, NB, N], BF16, tag="E1", bufs=2)
            rows0 = attrs.tile([P, NB], f32, tag="rows0")
            rows1 = attrs.tile([P, NB], f32, tag="rows1")
            accA = psum.tile([P, F], f32, tag="accA")
            accB = psum.tile([P, F], f32, tag="accB")
            accC = psum.tile([P, F], f32, tag="accC")
            accD = psum.tile([P, F], f32, tag="accD")
            pair_state[pc] = dict(E0=E0, E1=E1, rows0=rows0, rows1=rows1,
                                  accA=accA, accB=accB, accC=accC, accD=accD)

            # ----- first half: h0 scores+exps, then a batch of other work,
            # then attnv h1-batch of the previous pair -----
            emit_scores_batch(pc, 0, E0, rows0, list(range(NB)))
            if pc == 0:
                emit_kqvT_chunk(1)
                emit_kqvT_chunk(2)
            else:
                emit_attnv_batch(pc - 1, 1, list(range(NB)))
                emit_pair_finish(pc - 1)
                emit_evac(pc - 1, last=False)

            # ----- second half: h1 scores+exps + other work + attnv h0 -----
            emit_scores_batch(pc, 1, E1, rows1, list(range(NB)))
            if pc == 0:
                emit_kqvT_chunk(3)
                for nb0 in range(0, NB, 2):
                    emit_kqv_v_pair(nb0)
                nc.gpsimd.dma_start(
                    out=wo_sb, in_=woT.rearrange("(c p) o -> p c o", p=P))
            elif pc >= 2:
                emit_ypartial(pc - 2, list(range(NB)), with_dma=False)
            emit_attnv_batch(pc, 0, list(range(NB)))

        # =================== tail ===================
        pc = H // 2 - 1
        emit_attnv_batch(pc, 1, list(range(NB)))
        emit_pair_finish(pc)
        emit_ypartial(2, list(range(NB)), with_dma=False)
        emit_evac(pc, last=True)


_NC_CACHE = None


def build_nc():
    global _NC_CACHE
    if _NC_CACHE is None:
        nc = bacc.Bacc(trn_type="TRN2")
        _emit(nc)
        nc.finalize()
        _NC_CACHE = nc
    return _NC_CACHE


def _eff_weightT(weight, style):
    """Host: modulated+demodulated weight, transposed, bf16.
    weight [O, I] fp32, style [I] fp32 -> [I, O] bf16."""
    import ml_dtypes
    w = weight * style[None, :]
    w = w * (1.0 / np.sqrt((w * w).sum(axis=1) + EPS))[:, None]
    return np.ascontiguousarray(w.T.astype(ml_dtypes.bfloat16))


def make_in_maps(x, s, k_weight, k_aff_w, k_aff_b, o_weight, o_aff_w, o_aff_b):
    import ml_dtypes
    f = np.float32
    bf = ml_dtypes.bfloat16
    x = np.asarray(x, f)
    s = np.asarray(s, f)
    k_weight = np.asarray(k_weight, f)
    k_aff_w = np.asarray(k_aff_w, f)
    k_aff_b = np.asarray(k_aff_b, f)
    o_weight = np.asarray(o_weight, f)
    o_aff_w = np.asarray(o_aff_w, f)
    o_aff_b = np.asarray(o_aff_b, f)
    in_maps = []
    for b in range(B):
        style_k = s[b] @ k_aff_w.T + k_aff_b
        style_o = s[b] @ o_aff_w.T + o_aff_b
        in_maps.append({
            "xT": np.ascontiguousarray(x[b].T.astype(bf)),
            "wkT": _eff_weightT(k_weight, style_k),
            "woT": _eff_weightT(o_weight, style_o),
        })
    return in_maps


def kernel(x, s, k_weight, k_aff_w, k_aff_b, o_weight, o_aff_w, o_aff_b):
    assert x.shape == (B, N, F), x.shape
    nc = build_nc()
    in_maps = make_in_maps(x, s, k_weight, k_aff_w, k_aff_b,
                           o_weight, o_aff_w, o_aff_b)
    res = run_bass_kernel_spmd(nc, in_maps, list(range(B)))
    return np.stack([res.results[b]["y"] for b in range(B)], axis=0)
